# revision 14
# baseline (speedup 1.0000x reference)
"""ChannelAttention Trainium2 kernel.

Reference computation (per batch b, group o):
    p_mean[s, c] = mean over (h, w) of x[b, o, s, c, :, :]
    p_max[s, c]  = max  over (h, w) of x[b, o, s, c, :, :]
    out = sigmoid(relu(p_mean @ w1[o].T) @ w2[o].T + relu(p_max @ w1[o].T) @ w2[o].T)
    result[b, o, s, c, 0, 0] = out[s, c]

Strategy: data-parallel over batch B=8 -> one batch per NeuronCore (64 MiB
of x per core; the kernel is HBM-bandwidth bound on streaming x).

Per core, x[b] is viewed as [O*S*C, H*W] = [16384, 1024] and streamed in
2 MiB tiles of [128 partitions, 4*1024]. The vector engine computes the
row max (tensor_reduce) while the scalar engine computes the row mean
(activation Copy with scale=1/1024 and accum_out), so both reductions
hide under the DMA stream. 128 consecutive rows cover 2 s-values x 64
channels, so pooled results land as [partition = (s%2)*64 + c,
column = o*16 + s//2]. The tiny grouped MLP consumes that layout
directly by using block-diagonal duplicated weights
([[W.T, 0], [0, W.T]], built host-side): one 128x128x16 matmul per
(group, pooling path), relu, then two accumulating 16x128x128 matmuls
(mean + max paths summed in PSUM), sigmoid, and a strided store.
"""

import numpy as np

import concourse.bacc as bacc
import concourse.bass as bass
import concourse.mybir as mybir
import concourse.tile as tile
from concourse.bass_utils import run_bass_kernel_spmd

B, O, S, C, H, W = 8, 8, 32, 64, 32, 32
HID = C
HWSZ = H * W            # 1024 elements pooled per (b, o, s, c)
ROWS = O * S * C        # 16384 rows per core
RB = 128                # rows per partition block
T = ROWS // RB          # 128 row-blocks per core
JB = 4                  # row-blocks per stream tile (2 MiB DMAs)
NT = T // JB            # 32 stream tiles
SP = S // 2             # 16 pooled columns per group
N_CORES = 8

_CACHE = {}


def _build_nc():
    nc = bacc.Bacc(
        "TRN2", target_bir_lowering=False, debug=False, num_devices=N_CORES
    )
    x = nc.dram_tensor("x", [ROWS, HWSZ], mybir.dt.float32, kind="ExternalInput")
    wdup = nc.dram_tensor(
        "wdup", [128, 3 * O * 128], mybir.dt.float32, kind="ExternalInput"
    )
    out = nc.dram_tensor("out", [O * S, C], mybir.dt.float32, kind="ExternalOutput")

    fp32 = mybir.dt.float32
    AF = mybir.ActivationFunctionType

    with tile.TileContext(nc) as tc:
        with (
            tc.tile_pool(name="xp", bufs=8) as xp,
            tc.tile_pool(name="small", bufs=1) as sp,
            tc.tile_pool(name="psum1", bufs=1, space=bass.MemorySpace.PSUM) as pp1,
            tc.tile_pool(name="psum2", bufs=1, space=bass.MemorySpace.PSUM) as pp2,
        ):
            # Weight load issued first on the sync HWDGE queue so it expands
            # during the framework preamble / first x tile, when the DMA
            # engines still have slack (the GpSimd SWDGE path's descriptor-gen
            # latency pushed it into the middle of the x stream).
            wd = sp.tile([128, 3 * O * 128], fp32)
            nc.sync.dma_start(wd[:], wdup.ap())

            pooled_sum = sp.tile([128, T], fp32)
            pooled_max = sp.tile([128, T], fp32)
            junk = sp.tile([128, HWSZ], fp32)
            h_sb = sp.tile([128, O * 2 * SP], fp32)
            att = sp.tile([SP, O * 128], fp32)

            xv = x.ap().rearrange("(t p) f -> t p f", p=RB)
            ov = out.ap().rearrange("(o j r) c -> o j r c", o=O, j=SP, r=2)

            def mlp(o):
                w1s = wd[:, o * 128 : (o + 1) * 128]
                w1m = wd[:, O * 128 + o * 128 : O * 128 + (o + 1) * 128]
                w2b = wd[:, 2 * O * 128 + o * 128 : 2 * O * 128 + (o + 1) * 128]
                ps1m = pp1.tile([128, SP], fp32, tag="ps1m")
                ps1x = pp1.tile([128, SP], fp32, tag="ps1x")
                nc.tensor.matmul(ps1m[:], w1s, pooled_sum[:, o * SP : (o + 1) * SP])
                nc.tensor.matmul(ps1x[:], w1m, pooled_max[:, o * SP : (o + 1) * SP])
                hm = h_sb[:, o * 2 * SP : o * 2 * SP + SP]
                hx = h_sb[:, o * 2 * SP + SP : (o + 1) * 2 * SP]
                nc.scalar.activation(hm, ps1m[:], AF.Relu)
                nc.scalar.activation(hx, ps1x[:], AF.Relu)
                ps2 = pp2.tile([SP, 128], fp32, tag="ps2")
                nc.tensor.matmul(ps2[:], hm, w2b, start=True, stop=False)
                nc.tensor.matmul(ps2[:], hx, w2b, start=False, stop=True)
                ao = att[:, o * 128 : (o + 1) * 128]
                nc.scalar.activation(ao, ps2[:], AF.Sigmoid)
                nc.gpsimd.dma_start(ov[o], ao.rearrange("p (r c) -> p r c", r=2))

            # Chunk schedule: 2 MiB DMAs for the bulk of the stream, tapering
            # to 256 KiB at the end so the final reduce->MLP->store chain
            # starts on less data (shorter kernel tail).
            chunks = [4] * 30 + [2, 2, 2, 1, 1]
            assert sum(chunks) == T
            t0 = 0
            for i, jb in enumerate(chunks):
                xt = xp.tile([RB, JB, HWSZ], fp32, tag="xt")
                nc.sync.dma_start(
                    xt[:, :jb, :],
                    xv[t0 : t0 + jb].transpose([1, 0, 2]),
                )
                nc.vector.tensor_reduce(
                    pooled_max[:, t0 : t0 + jb],
                    xt[:, :jb, :],
                    axis=mybir.AxisListType.X,
                    op=mybir.AluOpType.max,
                )
                # Row sums (the 1/HWSZ mean scale is folded into the w1s
                # weight block host-side), split between the vector and
                # scalar engines so neither exceeds the DMA stream time:
                # DVE takes block 0 of every other full chunk plus the
                # final taper blocks (shortens the tail dependency chain).
                for j in range(jb):
                    t = t0 + j
                    if (i < 30 and j == 0 and i % 2 == 1) or (
                        i >= 32 and j == jb - 1 and t != 127
                    ):
                        nc.vector.tensor_reduce(
                            pooled_sum[:, t : t + 1],
                            xt[:, j, :],
                            axis=mybir.AxisListType.X,
                            op=mybir.AluOpType.add,
                        )
                    else:
                        nc.scalar.activation(
                            junk[:],
                            xt[:, j, :],
                            AF.Copy,
                            accum_out=pooled_sum[:, t : t + 1],
                        )
                # Group o's pooled columns [o*SP, (o+1)*SP) are complete once
                # row-blocks through t = (o+1)*SP - 1 are reduced; emit its MLP
                # as soon as that happens so it overlaps the remaining stream.
                done = t0 + jb
                for o in range(O):
                    if t0 < (o + 1) * SP <= done:
                        mlp(o)
                t0 = done

    nc.compile()
    return nc


def _build_wdup(w1, w2):
    # Three sections of 8 block-diagonal duplicated 128x128 matrices:
    # w1.T scaled by 1/HWSZ (consumes raw row sums -> mean path), w1.T
    # (max path), w2.T.
    wdup = np.zeros((128, 3 * O * 128), dtype=np.float32)
    for o in range(O):
        w1t = np.ascontiguousarray(w1[o].T)  # [C, HID]
        w2t = np.ascontiguousarray(w2[o].T)  # [HID, C]
        for sec, blk in ((0, w1t / HWSZ), (1, w1t), (2, w2t)):
            base = sec * O * 128 + o * 128
            wdup[0:64, base : base + 64] = blk
            wdup[64:128, base + 64 : base + 128] = blk
    return wdup


def kernel(x, w1, w2):
    if "nc" not in _CACHE:
        _CACHE["nc"] = _build_nc()
    nc = _CACHE["nc"]

    x = np.ascontiguousarray(x, dtype=np.float32).reshape(B, ROWS, HWSZ)
    wdup = _build_wdup(
        np.asarray(w1, dtype=np.float32), np.asarray(w2, dtype=np.float32)
    )
    in_maps = [{"x": x[b], "wdup": wdup} for b in range(B)]
    res = run_bass_kernel_spmd(nc, in_maps, core_ids=list(range(N_CORES)))
    out = np.stack([res.results[b]["out"] for b in range(B)])
    return out.reshape(B, O, S, C, 1, 1).astype(np.float32)


# revision 15
# speedup vs baseline: 1.1616x; 1.1616x over previous
"""ChannelAttention Trainium2 kernel.

Reference computation (per batch b, group o):
    p_mean[s, c] = mean over (h, w) of x[b, o, s, c, :, :]
    p_max[s, c]  = max  over (h, w) of x[b, o, s, c, :, :]
    out = sigmoid(relu(p_mean @ w1[o].T) @ w2[o].T + relu(p_max @ w1[o].T) @ w2[o].T)
    result[b, o, s, c, 0, 0] = out[s, c]

Strategy: data-parallel over batch B=8 -> one batch per NeuronCore (64 MiB
of x per core; the kernel is HBM-bandwidth bound on streaming x).

Per core, x[b] is viewed as [O*S*C, H*W] = [16384, 1024] and streamed in
2 MiB tiles of [128 partitions, 4*1024]. The vector engine computes the
row max (tensor_reduce) while the scalar engine computes the row mean
(activation Copy with scale=1/1024 and accum_out), so both reductions
hide under the DMA stream. 128 consecutive rows cover 2 s-values x 64
channels, so pooled results land as [partition = (s%2)*64 + c,
column = o*16 + s//2]. The tiny grouped MLP consumes that layout
directly by using block-diagonal duplicated weights
([[W.T, 0], [0, W.T]], built host-side): one 128x128x16 matmul per
(group, pooling path), relu, then two accumulating 16x128x128 matmuls
(mean + max paths summed in PSUM), sigmoid, and a strided store.
"""

import numpy as np

import concourse.bacc as bacc
import concourse.bass as bass
import concourse.mybir as mybir
import concourse.tile as tile
from concourse.bass_utils import run_bass_kernel_spmd

B, O, S, C, H, W = 8, 8, 32, 64, 32, 32
HID = C
HWSZ = H * W            # 1024 elements pooled per (b, o, s, c)
ROWS = O * S * C        # 16384 rows per core
RB = 128                # rows per partition block
T = ROWS // RB          # 128 row-blocks per core
JB = 4                  # row-blocks per stream tile (2 MiB DMAs)
NT = T // JB            # 32 stream tiles
SP = S // 2             # 16 pooled columns per group
N_CORES = 8

_CACHE = {}


def _build_nc():
    nc = bacc.Bacc(
        "TRN2", target_bir_lowering=False, debug=False, num_devices=N_CORES
    )
    x = nc.dram_tensor("x", [ROWS, HWSZ], mybir.dt.float32, kind="ExternalInput")
    wdup = nc.dram_tensor(
        "wdup", [128, 3 * O * 128], mybir.dt.float32, kind="ExternalInput"
    )
    out = nc.dram_tensor("out", [O * S, C], mybir.dt.float32, kind="ExternalOutput")

    fp32 = mybir.dt.float32
    AF = mybir.ActivationFunctionType

    with tile.TileContext(nc) as tc:
        with (
            tc.tile_pool(name="xp", bufs=8) as xp,
            tc.tile_pool(name="small", bufs=1) as sp,
            tc.tile_pool(name="psum1", bufs=1, space=bass.MemorySpace.PSUM) as pp1,
            tc.tile_pool(name="psum2", bufs=1, space=bass.MemorySpace.PSUM) as pp2,
        ):
            # Weight load issued first on the sync HWDGE queue so it expands
            # during the framework preamble / first x tile, when the DMA
            # engines still have slack (the GpSimd SWDGE path's descriptor-gen
            # latency pushed it into the middle of the x stream).
            wd = sp.tile([128, 3 * O * 128], fp32)
            nc.sync.dma_start(wd[:], wdup.ap())

            pooled_sum = sp.tile([128, T], fp32)
            pooled_max = sp.tile([128, T], fp32)
            junk = sp.tile([128, HWSZ], fp32)
            h_sb = sp.tile([128, O * 2 * SP], fp32)
            att = sp.tile([SP, O * 128], fp32)

            xv = x.ap().rearrange("(t p) f -> t p f", p=RB)
            ov = out.ap().rearrange("(o j r) c -> o j r c", o=O, j=SP, r=2)

            def mlp(o):
                w1s = wd[:, o * 128 : (o + 1) * 128]
                w1m = wd[:, O * 128 + o * 128 : O * 128 + (o + 1) * 128]
                w2b = wd[:, 2 * O * 128 + o * 128 : 2 * O * 128 + (o + 1) * 128]
                ps1m = pp1.tile([128, SP], fp32, tag="ps1m")
                ps1x = pp1.tile([128, SP], fp32, tag="ps1x")
                nc.tensor.matmul(ps1m[:], w1s, pooled_sum[:, o * SP : (o + 1) * SP])
                nc.tensor.matmul(ps1x[:], w1m, pooled_max[:, o * SP : (o + 1) * SP])
                hm = h_sb[:, o * 2 * SP : o * 2 * SP + SP]
                hx = h_sb[:, o * 2 * SP + SP : (o + 1) * 2 * SP]
                nc.scalar.activation(hm, ps1m[:], AF.Relu)
                nc.scalar.activation(hx, ps1x[:], AF.Relu)
                ps2 = pp2.tile([SP, 128], fp32, tag="ps2")
                nc.tensor.matmul(ps2[:], hm, w2b, start=True, stop=False)
                nc.tensor.matmul(ps2[:], hx, w2b, start=False, stop=True)
                ao = att[:, o * 128 : (o + 1) * 128]
                nc.scalar.activation(ao, ps2[:], AF.Sigmoid)
                nc.gpsimd.dma_start(ov[o], ao.rearrange("p (r c) -> p r c", r=2))

            # Group 7 - whose pooled columns complete last - uses a
            # column-split MLP emitted across the final taper chunks, so only
            # ~1 column of FC1/FC2 work trails the last byte.
            h7 = sp.tile([128, 2 * SP], fp32)
            g7 = {}

            def g7_fc1(c0, c1):
                if "ps1m" not in g7:
                    g7["ps1m"] = pp1.tile([128, SP], fp32, tag="g7m", name="g7m")
                    g7["ps1x"] = pp1.tile([128, SP], fp32, tag="g7x", name="g7x")
                pc = slice(112 + c0, 112 + c1)
                w1s7 = wd[:, 7 * 128 : 8 * 128]
                w1m7 = wd[:, O * 128 + 7 * 128 : O * 128 + 8 * 128]
                nc.tensor.matmul(g7["ps1x"][:, c0:c1], w1m7, pooled_max[:, pc])
                nc.tensor.matmul(g7["ps1m"][:, c0:c1], w1s7, pooled_sum[:, pc])

            def g7_fc2(c0, c1):
                # relu the new column range, then FC2 rows [c0, c1). Matmul
                # PSUM outputs must start at partition 0/32/64, so each piece
                # gets its own tile; activation outputs have the same base
                # restriction, so the last piece's sigmoid goes through a
                # partition-0 tile (the DMA store has no such restriction).
                nc.scalar.activation(h7[:, c0:c1], g7["ps1m"][:, c0:c1], AF.Relu)
                nc.scalar.activation(
                    h7[:, SP + c0 : SP + c1], g7["ps1x"][:, c0:c1], AF.Relu
                )
                ps2 = pp2.tile([c1 - c0, 128], fp32, tag=f"g7p{c0}", name=f"g7p{c0}")
                w2b7 = wd[:, 2 * O * 128 + 7 * 128 : 2 * O * 128 + 8 * 128]
                nc.tensor.matmul(ps2[:], h7[:, c0:c1], w2b7, start=True, stop=False)
                nc.tensor.matmul(
                    ps2[:], h7[:, SP + c0 : SP + c1], w2b7, start=False, stop=True
                )
                if c0 == 0:
                    ao = att[c0:c1, 7 * 128 : 8 * 128]
                else:
                    ao = sp.tile([c1 - c0, 128], fp32, name=f"att7_{c0}")
                nc.scalar.activation(ao, ps2[:], AF.Sigmoid)
                nc.gpsimd.dma_start(
                    ov[7][c0:c1], ao.rearrange("p (r c) -> p r c", r=2)
                )

            # Chunk schedule: 2 MiB DMAs for the bulk of the stream, tapering
            # to 256 KiB at the end so the final reduce->MLP->store chain
            # starts on less data (shorter kernel tail).
            chunks = [4] * 30 + [2, 2, 2, 1, 1]
            assert sum(chunks) == T
            t0 = 0
            for i, jb in enumerate(chunks):
                xt = xp.tile([RB, JB, HWSZ], fp32, tag="xt")
                nc.sync.dma_start(
                    xt[:, :jb, :],
                    xv[t0 : t0 + jb].transpose([1, 0, 2]),
                )
                nc.vector.tensor_reduce(
                    pooled_max[:, t0 : t0 + jb],
                    xt[:, :jb, :],
                    axis=mybir.AxisListType.X,
                    op=mybir.AluOpType.max,
                )
                # Row sums (the 1/HWSZ mean scale is folded into the w1s
                # weight block host-side), split between the vector and
                # scalar engines so neither exceeds the DMA stream time:
                # DVE takes block 0 of every other full chunk plus the
                # final taper blocks (shortens the tail dependency chain).
                for j in range(jb):
                    t = t0 + j
                    if (i < 30 and j == 0 and i % 2 == 1) or (
                        i >= 32 and j == jb - 1 and t != 127
                    ):
                        nc.vector.tensor_reduce(
                            pooled_sum[:, t : t + 1],
                            xt[:, j, :],
                            axis=mybir.AxisListType.X,
                            op=mybir.AluOpType.add,
                        )
                    else:
                        nc.scalar.activation(
                            junk[:],
                            xt[:, j, :],
                            AF.Copy,
                            accum_out=pooled_sum[:, t : t + 1],
                        )
                # Group o's pooled columns [o*SP, (o+1)*SP) are complete once
                # row-blocks through t = (o+1)*SP - 1 are reduced; emit its MLP
                # as soon as that happens so it overlaps the remaining stream.
                done = t0 + jb
                for o in range(7):
                    if t0 < (o + 1) * SP <= done:
                        mlp(o)
                # Group 7: FC1 for freshly completed columns; FC2 for rows
                # 0-14 once available, the last row at the end.
                nc0, nc1 = max(t0, 112) - 112, max(done, 112) - 112
                if nc1 > nc0:
                    g7_fc1(nc0, nc1)
                    if nc0 < 15 <= nc1:
                        g7_fc2(0, 15)
                    if nc1 == 16:
                        g7_fc2(15, 16)
                t0 = done

    nc.compile()
    return nc


def _build_wdup(w1, w2):
    # Three sections of 8 block-diagonal duplicated 128x128 matrices:
    # w1.T scaled by 1/HWSZ (consumes raw row sums -> mean path), w1.T
    # (max path), w2.T.
    wdup = np.zeros((128, 3 * O * 128), dtype=np.float32)
    for o in range(O):
        w1t = np.ascontiguousarray(w1[o].T)  # [C, HID]
        w2t = np.ascontiguousarray(w2[o].T)  # [HID, C]
        for sec, blk in ((0, w1t / HWSZ), (1, w1t), (2, w2t)):
            base = sec * O * 128 + o * 128
            wdup[0:64, base : base + 64] = blk
            wdup[64:128, base + 64 : base + 128] = blk
    return wdup


def kernel(x, w1, w2):
    if "nc" not in _CACHE:
        _CACHE["nc"] = _build_nc()
    nc = _CACHE["nc"]

    x = np.ascontiguousarray(x, dtype=np.float32).reshape(B, ROWS, HWSZ)
    wdup = _build_wdup(
        np.asarray(w1, dtype=np.float32), np.asarray(w2, dtype=np.float32)
    )
    in_maps = [{"x": x[b], "wdup": wdup} for b in range(B)]
    res = run_bass_kernel_spmd(nc, in_maps, core_ids=list(range(N_CORES)))
    out = np.stack([res.results[b]["out"] for b in range(B)])
    return out.reshape(B, O, S, C, 1, 1).astype(np.float32)


# revision 16
# speedup vs baseline: 1.1648x; 1.0028x over previous
"""ChannelAttention Trainium2 kernel.

Reference computation (per batch b, group o):
    p_mean[s, c] = mean over (h, w) of x[b, o, s, c, :, :]
    p_max[s, c]  = max  over (h, w) of x[b, o, s, c, :, :]
    out = sigmoid(relu(p_mean @ w1[o].T) @ w2[o].T + relu(p_max @ w1[o].T) @ w2[o].T)
    result[b, o, s, c, 0, 0] = out[s, c]

Strategy: data-parallel over batch B=8 -> one batch per NeuronCore (64 MiB
of x per core; the kernel is HBM-bandwidth bound on streaming x).

Per core, x[b] is viewed as [O*S*C, H*W] = [16384, 1024] and streamed in
2 MiB tiles of [128 partitions, 4*1024]. The vector engine computes the
row max (tensor_reduce) while the scalar engine computes the row mean
(activation Copy with scale=1/1024 and accum_out), so both reductions
hide under the DMA stream. 128 consecutive rows cover 2 s-values x 64
channels, so pooled results land as [partition = (s%2)*64 + c,
column = o*16 + s//2]. The tiny grouped MLP consumes that layout
directly by using block-diagonal duplicated weights
([[W.T, 0], [0, W.T]], built host-side): one 128x128x16 matmul per
(group, pooling path), relu, then two accumulating 16x128x128 matmuls
(mean + max paths summed in PSUM), sigmoid, and a strided store.
"""

import numpy as np

import concourse.bacc as bacc
import concourse.bass as bass
import concourse.mybir as mybir
import concourse.tile as tile
from concourse.bass_utils import run_bass_kernel_spmd

B, O, S, C, H, W = 8, 8, 32, 64, 32, 32
HID = C
HWSZ = H * W            # 1024 elements pooled per (b, o, s, c)
ROWS = O * S * C        # 16384 rows per core
RB = 128                # rows per partition block
T = ROWS // RB          # 128 row-blocks per core
JB = 4                  # row-blocks per stream tile (2 MiB DMAs)
NT = T // JB            # 32 stream tiles
SP = S // 2             # 16 pooled columns per group
N_CORES = 8

_CACHE = {}


def _build_nc():
    nc = bacc.Bacc(
        "TRN2", target_bir_lowering=False, debug=False, num_devices=N_CORES
    )
    x = nc.dram_tensor("x", [ROWS, HWSZ], mybir.dt.float32, kind="ExternalInput")
    wdup = nc.dram_tensor(
        "wdup", [128, 3 * O * 128], mybir.dt.float32, kind="ExternalInput"
    )
    out = nc.dram_tensor("out", [O * S, C], mybir.dt.float32, kind="ExternalOutput")

    fp32 = mybir.dt.float32
    AF = mybir.ActivationFunctionType

    with tile.TileContext(nc) as tc:
        with (
            tc.tile_pool(name="xp", bufs=10) as xp,
            tc.tile_pool(name="small", bufs=1) as sp,
            tc.tile_pool(name="psum1", bufs=1, space=bass.MemorySpace.PSUM) as pp1,
            tc.tile_pool(name="psum2", bufs=1, space=bass.MemorySpace.PSUM) as pp2,
        ):
            # Weight load issued first on the sync HWDGE queue so it expands
            # during the framework preamble / first x tile, when the DMA
            # engines still have slack (the GpSimd SWDGE path's descriptor-gen
            # latency pushed it into the middle of the x stream).
            wd = sp.tile([128, 3 * O * 128], fp32)
            nc.sync.dma_start(wd[:], wdup.ap())

            pooled_sum = sp.tile([128, T], fp32)
            pooled_max = sp.tile([128, T], fp32)
            junk = sp.tile([128, HWSZ], fp32)
            h_sb = sp.tile([128, O * 2 * SP], fp32)
            att = sp.tile([SP, O * 128], fp32)

            xv = x.ap().rearrange("(t p) f -> t p f", p=RB)
            ov = out.ap().rearrange("(o j r) c -> o j r c", o=O, j=SP, r=2)

            def mlp(o):
                w1s = wd[:, o * 128 : (o + 1) * 128]
                w1m = wd[:, O * 128 + o * 128 : O * 128 + (o + 1) * 128]
                w2b = wd[:, 2 * O * 128 + o * 128 : 2 * O * 128 + (o + 1) * 128]
                ps1m = pp1.tile([128, SP], fp32, tag="ps1m")
                ps1x = pp1.tile([128, SP], fp32, tag="ps1x")
                nc.tensor.matmul(ps1m[:], w1s, pooled_sum[:, o * SP : (o + 1) * SP])
                nc.tensor.matmul(ps1x[:], w1m, pooled_max[:, o * SP : (o + 1) * SP])
                hm = h_sb[:, o * 2 * SP : o * 2 * SP + SP]
                hx = h_sb[:, o * 2 * SP + SP : (o + 1) * 2 * SP]
                nc.scalar.activation(hm, ps1m[:], AF.Relu)
                nc.scalar.activation(hx, ps1x[:], AF.Relu)
                ps2 = pp2.tile([SP, 128], fp32, tag="ps2")
                nc.tensor.matmul(ps2[:], hm, w2b, start=True, stop=False)
                nc.tensor.matmul(ps2[:], hx, w2b, start=False, stop=True)
                ao = att[:, o * 128 : (o + 1) * 128]
                nc.scalar.activation(ao, ps2[:], AF.Sigmoid)
                nc.gpsimd.dma_start(ov[o], ao.rearrange("p (r c) -> p r c", r=2))

            # Group 7 - whose pooled columns complete last - uses a
            # column-split MLP emitted across the final taper chunks, so only
            # ~1 column of FC1/FC2 work trails the last byte.
            h7 = sp.tile([128, 2 * SP], fp32)
            g7 = {}

            def g7_fc1(c0, c1):
                if "ps1m" not in g7:
                    g7["ps1m"] = pp1.tile([128, SP], fp32, tag="g7m", name="g7m")
                    g7["ps1x"] = pp1.tile([128, SP], fp32, tag="g7x", name="g7x")
                pc = slice(112 + c0, 112 + c1)
                w1s7 = wd[:, 7 * 128 : 8 * 128]
                w1m7 = wd[:, O * 128 + 7 * 128 : O * 128 + 8 * 128]
                nc.tensor.matmul(g7["ps1x"][:, c0:c1], w1m7, pooled_max[:, pc])
                nc.tensor.matmul(g7["ps1m"][:, c0:c1], w1s7, pooled_sum[:, pc])

            def g7_fc2(c0, c1):
                # relu the new column range, then FC2 rows [c0, c1). Matmul
                # PSUM outputs must start at partition 0/32/64, so each piece
                # gets its own tile; activation outputs have the same base
                # restriction, so the last piece's sigmoid goes through a
                # partition-0 tile (the DMA store has no such restriction).
                nc.scalar.activation(h7[:, c0:c1], g7["ps1m"][:, c0:c1], AF.Relu)
                nc.scalar.activation(
                    h7[:, SP + c0 : SP + c1], g7["ps1x"][:, c0:c1], AF.Relu
                )
                ps2 = pp2.tile([c1 - c0, 128], fp32, tag=f"g7p{c0}", name=f"g7p{c0}")
                w2b7 = wd[:, 2 * O * 128 + 7 * 128 : 2 * O * 128 + 8 * 128]
                nc.tensor.matmul(ps2[:], h7[:, c0:c1], w2b7, start=True, stop=False)
                nc.tensor.matmul(
                    ps2[:], h7[:, SP + c0 : SP + c1], w2b7, start=False, stop=True
                )
                if c0 == 0:
                    ao = att[c0:c1, 7 * 128 : 8 * 128]
                else:
                    ao = sp.tile([c1 - c0, 128], fp32, name=f"att7_{c0}")
                nc.scalar.activation(ao, ps2[:], AF.Sigmoid)
                nc.gpsimd.dma_start(
                    ov[7][c0:c1], ao.rearrange("p (r c) -> p r c", r=2)
                )

            # Chunk schedule: 2 MiB DMAs for the bulk of the stream, tapering
            # to 256 KiB at the end so the final reduce->MLP->store chain
            # starts on less data (shorter kernel tail).
            chunks = [4] * 30 + [2, 2, 2, 1, 1]
            assert sum(chunks) == T
            t0 = 0
            for i, jb in enumerate(chunks):
                xt = xp.tile([RB, JB, HWSZ], fp32, tag="xt")
                nc.sync.dma_start(
                    xt[:, :jb, :],
                    xv[t0 : t0 + jb].transpose([1, 0, 2]),
                )
                nc.vector.tensor_reduce(
                    pooled_max[:, t0 : t0 + jb],
                    xt[:, :jb, :],
                    axis=mybir.AxisListType.X,
                    op=mybir.AluOpType.max,
                )
                # Row sums (the 1/HWSZ mean scale is folded into the w1s
                # weight block host-side), split between the vector and
                # scalar engines so neither exceeds the DMA stream time:
                # DVE takes block 0 of every other full chunk plus the
                # final taper blocks (shortens the tail dependency chain).
                for j in range(jb):
                    t = t0 + j
                    if (i < 30 and j == 0 and i % 2 == 1) or t == 127:
                        nc.vector.tensor_reduce(
                            pooled_sum[:, t : t + 1],
                            xt[:, j, :],
                            axis=mybir.AxisListType.X,
                            op=mybir.AluOpType.add,
                        )
                    else:
                        nc.scalar.activation(
                            junk[:],
                            xt[:, j, :],
                            AF.Copy,
                            accum_out=pooled_sum[:, t : t + 1],
                        )
                # Group o's pooled columns [o*SP, (o+1)*SP) are complete once
                # row-blocks through t = (o+1)*SP - 1 are reduced; emit its MLP
                # as soon as that happens so it overlaps the remaining stream.
                done = t0 + jb
                for o in range(7):
                    if t0 < (o + 1) * SP <= done:
                        mlp(o)
                # Group 7: FC1 for freshly completed columns; FC2 for rows
                # 0-14 once available, the last row at the end.
                nc0, nc1 = max(t0, 112) - 112, max(done, 112) - 112
                if nc1 > nc0:
                    g7_fc1(nc0, nc1)
                    if nc0 < 14 <= nc1:
                        g7_fc2(0, 14)
                    if nc1 == 16:
                        g7_fc2(14, 16)
                t0 = done

    nc.compile()
    return nc


def _build_wdup(w1, w2):
    # Three sections of 8 block-diagonal duplicated 128x128 matrices:
    # w1.T scaled by 1/HWSZ (consumes raw row sums -> mean path), w1.T
    # (max path), w2.T.
    wdup = np.zeros((128, 3 * O * 128), dtype=np.float32)
    for o in range(O):
        w1t = np.ascontiguousarray(w1[o].T)  # [C, HID]
        w2t = np.ascontiguousarray(w2[o].T)  # [HID, C]
        for sec, blk in ((0, w1t / HWSZ), (1, w1t), (2, w2t)):
            base = sec * O * 128 + o * 128
            wdup[0:64, base : base + 64] = blk
            wdup[64:128, base + 64 : base + 128] = blk
    return wdup


def kernel(x, w1, w2):
    if "nc" not in _CACHE:
        _CACHE["nc"] = _build_nc()
    nc = _CACHE["nc"]

    x = np.ascontiguousarray(x, dtype=np.float32).reshape(B, ROWS, HWSZ)
    wdup = _build_wdup(
        np.asarray(w1, dtype=np.float32), np.asarray(w2, dtype=np.float32)
    )
    in_maps = [{"x": x[b], "wdup": wdup} for b in range(B)]
    res = run_bass_kernel_spmd(nc, in_maps, core_ids=list(range(N_CORES)))
    out = np.stack([res.results[b]["out"] for b in range(B)])
    return out.reshape(B, O, S, C, 1, 1).astype(np.float32)
